# revision 1
# baseline (speedup 1.0000x reference)
"""RNN-T joint network kernel for Trainium2 (8 NeuronCores, SPMD).

out[b,t,u,v] = (enc[b,t] @ W_enc.T)[v] + (dec[b,u] @ W_dec.T)[v]

Shapes: enc (4,512,512), dec (4,128,512), W (1024,1024) -> out (4,512,128,1024) f32 (1 GiB).

Strategy: shard T across the 8 cores (64 rows each). The 1 GiB output write
is the roofline (~375us/core at ~358 GB/s HBM-per-NC), so the kernel keeps
compute far under that:
  - host pre-transposes all inputs to contraction-major, so the small
    projection matmuls need no on-device transposes (fp32, exact).
  - the (T,U,V) broadcast-add is done in a v-on-partitions layout where the
    encoder term is a per-partition scalar -> DVE tensor_scalar runs at
    2 elem/cycle/lane fp32 (vs 1x for tensor_tensor), with ~30% of tiles
    offloaded to the scalar engine (Identity activation with AP bias).
  - output is written in device layout (B, V, T_loc, U) so every DMA line is
    8 KB contiguous; the host transposes back when gathering.
"""

import sys

if "/opt/trn_rl_repo" not in sys.path:
    sys.path.insert(0, "/opt/trn_rl_repo")

import numpy as np

# Problem shape (hardcoded per contract)
B, T, U, D, V = 4, 512, 128, 512, 1024
N_CORES = 8
P = 128

T_LOC = T // N_CORES          # 64 t-rows per core
TOK = B * T_LOC               # 256 (b,t) rows per core
KT = D // P                   # 4 contraction tiles
VT = V // P                   # 8 v tiles
T_CHUNK = 32                  # t rows per staging tile / output DMA
N_TCH = T_LOC // T_CHUNK      # 4 chunks
BU = B * U                    # 512

_CACHE: dict = {}


def _emit(tc, aps, mybir, act_frac_num=3, act_frac_den=10):
    """Emit the per-core Tile program.

    aps: dict with encT (D,TOK), decT (D,BU), wencT (D,V), wdecT (D,V),
    out (B, VT, P, N_TCH, T_CHUNK*U).
    """
    from contextlib import ExitStack

    nc = tc.nc
    f32 = mybir.dt.float32
    encT, decT, wencT, wdecT, out = (
        aps["encT"], aps["decT"], aps["wencT"], aps["wdecT"], aps["out"],
    )
    b_, vt, p_, ntch, chunk = out.shape
    tok_loc = encT.shape[1] // b_      # t rows per core
    bu = decT.shape[1]
    u_ = bu // b_
    kt = encT.shape[0] // P
    t_chunk = chunk // u_

    with ExitStack() as ctx:
        const = ctx.enter_context(tc.tile_pool(name="const", bufs=1))
        psum = ctx.enter_context(tc.tile_pool(name="psum", bufs=4, space="PSUM"))
        stage = ctx.enter_context(tc.tile_pool(name="stage", bufs=4))

        # --- input loads, critical-path first ---
        # Each logical tensor is loaded with ONE large DMA (k-tiles packed
        # side-by-side in the SBUF free dim) -- large transfers keep the
        # descriptor overhead near zero. Order: the ~2 MB "minimal set"
        # (m=0 weight columns + dec + enc) first, so small early (b=0-only)
        # m=0 projections can start the output stream at ~17us while the
        # remaining 3.5 MB of weight columns stream in underneath.
        def load(src, lo, hi, tag):
            """One DMA: src[:, lo:hi] (D x w) -> SBUF [P, kt*w], free=(k, col)."""
            w = hi - lo
            t = const.tile([P, kt * w], f32, tag=tag)
            nc.sync.dma_start(
                out=t[:].rearrange("p (k c) -> p k c", c=w),
                in_=src[:, lo:hi].rearrange("(k p) c -> p k c", p=P),
            )
            return t

        wdec_m0 = load(wdecT, 0, P, "wdec0")     # [P, kt*128]
        dec_t = load(decT, 0, bu, "dec")         # [P, kt*512]
        wenc_m0 = load(wencT, 0, P, "wenc0")     # [P, kt*128]
        enc_t = load(encT, 0, tokw_g := encT.shape[1], "enc")  # [P, kt*tokw]

        def project(mm_groups, width, tag, on_vector):
            """mm_groups: (lhs_tile, lhs_w, lhs_lo, rhs_tile, rhs_w, rhs_lo, rhs_n, out_lo)."""
            ps = psum.tile([P, width], f32, tag="ps" + tag[0])
            for lhs, lhs_w, lhs_lo, rhs, rhs_w, rhs_lo, rhs_n, out_lo in mm_groups:
                for k in range(kt):
                    nc.tensor.matmul(
                        ps[:, out_lo : out_lo + rhs_n],
                        lhsT=lhs[:, k * lhs_w + lhs_lo : k * lhs_w + lhs_lo + P],
                        rhs=rhs[:, k * rhs_w + rhs_lo : k * rhs_w + rhs_lo + rhs_n],
                        start=(k == 0),
                        stop=(k == kt - 1),
                    )
            sb = const.tile([P, width], f32, tag=tag)
            if on_vector:
                nc.vector.tensor_copy(out=sb[:], in_=ps[:])
            else:
                nc.scalar.activation(sb[:], ps[:], mybir.ActivationFunctionType.Copy)
            return sb

        # early (b=0-only) m=0 projections gate the first output chunks
        dproj0a = project([(wdec_m0, P, 0, dec_t, bu, 0, u_, 0)], u_, "dproj0a", True)
        eproj0a = project(
            [(wenc_m0, P, 0, enc_t, tokw_g, 0, tok_loc, 0)], tok_loc, "eproj0a", False
        )

        def emit_chunk(S_dst, dslice, eproj_tile, tok0, opi):
            for tt in range(t_chunk):
                col = eproj_tile[:, tok0 + tt : tok0 + tt + 1]
                dst = S_dst[:, tt * u_ : (tt + 1) * u_]
                if (opi * act_frac_num) % act_frac_den < act_frac_num:
                    nc.scalar.activation(
                        dst, dslice, mybir.ActivationFunctionType.Identity, bias=col
                    )
                else:
                    nc.vector.tensor_scalar_add(out=dst, in0=dslice, scalar1=col)
                opi += 1
            return opi

        opi = 0
        for tch in range(ntch):  # m=0, b=0 from the early projections
            S = stage.tile([P, chunk], f32, tag="stage")
            opi = emit_chunk(S, dproj0a[:, :u_], eproj0a, tch * t_chunk, opi)
            nc.sync.dma_start(out=out[0, 0, :, tch, :], in_=S[:])

        # --- remaining weight columns + full projections ---
        wr_w = wdecT.shape[1] - P
        wdec_r = load(wdecT, P, wdecT.shape[1], "wdecr")   # [P, kt*896]
        wenc_r = load(wencT, P, wencT.shape[1], "wencr")

        dproj, eproj = [], []
        tokw = encT.shape[1]
        for m in range(vt):
            wd = (wdec_m0, P, 0) if m == 0 else (wdec_r, wr_w, (m - 1) * P)
            we = (wenc_m0, P, 0) if m == 0 else (wenc_r, wr_w, (m - 1) * P)
            dproj.append(
                project(
                    [(wd[0], wd[1], wd[2], dec_t, bu, 0, bu, 0)], bu, f"dproj{m}", True
                )
            )
            eproj.append(
                project(
                    [(we[0], we[1], we[2], enc_t, tokw, 0, tokw, 0)],
                    tokw,
                    f"eproj{m}",
                    False,
                )
            )

        # --- broadcast-add main loop (m=0/b=0 already emitted above) ---
        for m in range(vt):
            for b in range(b_):
                if m == 0 and b == 0:
                    continue
                dslice = dproj[m][:, b * u_ : (b + 1) * u_]
                for tch in range(ntch):
                    S = stage.tile([P, chunk], f32, tag="stage")
                    opi = emit_chunk(S, dslice, eproj[m], b * tok_loc + tch * t_chunk, opi)
                    nc.sync.dma_start(out=out[b, m, :, tch, :], in_=S[:])


def build_bass(num_devices=N_CORES):
    """Build + compile the SPMD Bass program (cached)."""
    key = ("nc", num_devices)
    if key in _CACHE:
        return _CACHE[key]
    import concourse.bacc as bacc
    import concourse.tile as tile
    from concourse import mybir

    nc = bacc.Bacc(
        "TRN2",
        target_bir_lowering=False,
        debug=False,
        num_devices=num_devices,
    )
    f32 = mybir.dt.float32
    aps = {
        "encT": nc.dram_tensor("encT", [D, TOK], f32, kind="ExternalInput").ap(),
        "decT": nc.dram_tensor("decT", [D, BU], f32, kind="ExternalInput").ap(),
        "wencT": nc.dram_tensor("wencT", [D, V], f32, kind="ExternalInput").ap(),
        "wdecT": nc.dram_tensor("wdecT", [D, V], f32, kind="ExternalInput").ap(),
        "out": nc.dram_tensor(
            "out", [B, VT, P, N_TCH, T_CHUNK * U], f32, kind="ExternalOutput"
        ).ap(),
    }
    with tile.TileContext(nc) as tc:
        _emit(tc, aps, mybir)
    nc.compile()
    _CACHE[key] = nc
    return nc


def make_in_maps(encoder_outputs, decoder_outputs, fc_weight):
    enc = np.ascontiguousarray(encoder_outputs, dtype=np.float32)
    dec = np.ascontiguousarray(decoder_outputs, dtype=np.float32)
    w = np.ascontiguousarray(fc_weight, dtype=np.float32)
    decT = np.ascontiguousarray(dec.reshape(BU, D).T)
    wencT = np.ascontiguousarray(w[:, :D].T)
    wdecT = np.ascontiguousarray(w[:, D:].T)
    in_maps = []
    for c in range(N_CORES):
        enc_c = enc[:, c * T_LOC : (c + 1) * T_LOC, :].reshape(TOK, D)
        in_maps.append(
            {
                "encT": np.ascontiguousarray(enc_c.T),
                "decT": decT,
                "wencT": wencT,
                "wdecT": wdecT,
            }
        )
    return in_maps


def assemble(results):
    """results: list of per-core {"out": (B,VT,P,N_TCH,T_CHUNK*U)} -> (B,T,U,V)."""
    full = np.empty((B, T, U, V), dtype=np.float32)
    for c in range(N_CORES):
        arr = results[c]["out"].reshape(B, V, T_LOC, U)
        full[:, c * T_LOC : (c + 1) * T_LOC] = arr.transpose(0, 2, 3, 1)
    return full


def kernel(encoder_outputs, decoder_outputs, fc_weight):
    from concourse.bass_utils import run_bass_kernel_spmd

    nc = build_bass()
    in_maps = make_in_maps(encoder_outputs, decoder_outputs, fc_weight)
    res = run_bass_kernel_spmd(nc, in_maps, list(range(N_CORES)))
    return assemble(res.results)



# revision 3
# speedup vs baseline: 1.9001x; 1.9001x over previous
"""RNN-T joint network kernel for Trainium2 (8 NeuronCores, SPMD).

out[b,t,u,v] = (enc[b,t] @ W_enc.T)[v] + (dec[b,u] @ W_dec.T)[v]

Shapes: enc (4,512,512), dec (4,128,512), W (1024,1024) -> out (4,512,128,1024).

v2 strategy (fp16 output, rel tolerance is 2e-2 so fp16 store is free accuracy-wise):
  - shard T across the 8 cores (64 rows each). Inputs and output in fp16:
    the per-core HBM write drops from 134 MB to 67 MB -> ~187us roofline/core.
  - host pre-transposes inputs to contraction-major fp16; projections are
    fp16 matmuls accumulated in fp32 PSUM, copied to SBUF as fp16 by ACT.
  - broadcast-add as big DVE tensor_tensor fp16 ops [128, 8192] in 2x_1p
    mode (4.33us each): the e-term is pre-replicated only 8x into a small
    e_small[v,(b,t,j8)] tile so BOTH operands keep innermost stride +1
    (access pattern (t, rep16, j8)); the d-term broadcasts along t (outer
    stride 0). DVE total ~150us, under the DMA floor.
  - output written in device layout (B, VT, 128, T_loc*U) fp16, one 2 MB
    DMA per (b, m); host upconverts + transposes when gathering.
"""

import sys

if "/opt/trn_rl_repo" not in sys.path:
    sys.path.insert(0, "/opt/trn_rl_repo")

import numpy as np

# Problem shape (hardcoded per contract)
B, T, U, D, V = 4, 512, 128, 512, 1024
N_CORES = 8
P = 128

T_LOC = T // N_CORES          # 64 t-rows per core
TOK = B * T_LOC               # 256 (b,t) rows per core
KT = D // P                   # 4 contraction tiles
VT = V // P                   # 8 v tiles
BU = B * U                    # 512
CHUNK = T_LOC * U             # 8192 free elems per (b, m) chunk
J = 8                         # e_small replication run
R = U // J                    # 16

_CACHE: dict = {}


def _emit(tc, aps, mybir):
    from contextlib import ExitStack

    nc = tc.nc
    f16 = mybir.dt.float16
    f32 = mybir.dt.float32
    encT, decT, wencT, wdecT, out = (
        aps["encT"], aps["decT"], aps["wencT"], aps["wdecT"], aps["out"],
    )

    with ExitStack() as ctx:
        const = ctx.enter_context(tc.tile_pool(name="const", bufs=1))
        psum = ctx.enter_context(tc.tile_pool(name="psum", bufs=4, space="PSUM"))
        esm = ctx.enter_context(tc.tile_pool(name="esm", bufs=3))
        stage = ctx.enter_context(tc.tile_pool(name="stage", bufs=6))

        # --- input loads, critical-path first ---
        def load(src, lo, hi, tag):
            """One DMA: src[:, lo:hi] (D x w) -> SBUF [P, kt*w], free=(k, col)."""
            w = hi - lo
            t = const.tile([P, KT * w], f16, tag=tag)
            nc.sync.dma_start(
                out=t[:].rearrange("p (k c) -> p k c", c=w),
                in_=src[:, lo:hi].rearrange("(k p) c -> p k c", p=P),
            )
            return t

        wenc_m0 = load(wencT, 0, P, "wenc0")       # [P, 4*128]
        enc_t = load(encT, 0, TOK, "enc")          # [P, 4*256]
        wdec_m0 = load(wdecT, 0, P, "wdec0")       # [P, 4*128]
        dec_t = load(decT, 0, BU, "dec")           # [P, 4*512]
        wenc_r = load(wencT, P, V, "wencr")        # [P, 4*896]
        wdec_r = load(wdecT, P, V, "wdecr")

        def project(lhs, lhs_w, lhs_lo, rhs, rhs_w, n, tag):
            """psum[P, n] = sum_k lhs[:, k*lhs_w+lhs_lo : +128].T @ rhs[:, k*rhs_w : +n];
            ACT-copy to SBUF fp16."""
            ps = psum.tile([P, n], f32, tag="ps" + tag[0])
            for k in range(KT):
                nc.tensor.matmul(
                    ps[:],
                    lhsT=lhs[:, k * lhs_w + lhs_lo : k * lhs_w + lhs_lo + P],
                    rhs=rhs[:, k * rhs_w : k * rhs_w + n],
                    start=(k == 0),
                    stop=(k == KT - 1),
                )
            sb = const.tile([P, n], f16, tag=tag)
            nc.scalar.activation(sb[:], ps[:], mybir.ActivationFunctionType.Copy)
            return sb

        for m in range(VT):
            if m == 0:
                we, we_w, we_lo = wenc_m0, P, 0
                wd, wd_w, wd_lo = wdec_m0, P, 0
            else:
                we, we_w, we_lo = wenc_r, V - P, (m - 1) * P
                wd, wd_w, wd_lo = wdec_r, V - P, (m - 1) * P

            eproj = project(we, we_w, we_lo, enc_t, TOK, TOK, f"eproj{m}")   # [P,(b,t)]
            dproj = project(wd, wd_w, wd_lo, dec_t, BU, BU, f"dproj{m}")     # [P,(b,u)]

            # e_small_m [P, (b, t, j)]: e replicated J times along j (DVE 2x copy)
            es = esm.tile([P, B * T_LOC * J], f16, tag="esmall")
            nc.vector.tensor_copy(
                out=es[:].rearrange("p (b t j) -> p b t j", t=T_LOC, j=J),
                in_=eproj[:].rearrange("p (b t) -> p b t", t=T_LOC)[:, :, :, None]
                .to_broadcast((P, B, T_LOC, J)),
            )

            for b in range(B):
                # stage[v, (t, r, j)] = e_small[v, (t, j)] bcast over r
                #                      + dproj[v, (r, j)] bcast over t   (all stride+1 innermost)
                S = stage.tile([P, CHUNK], f16, tag="stage")
                e_in = (
                    es[:, b * T_LOC * J : (b + 1) * T_LOC * J]
                    .rearrange("p (t j) -> p t j", j=J)[:, :, None, :]
                    .to_broadcast((P, T_LOC, R, J))
                )
                d_in = (
                    dproj[:, b * U : (b + 1) * U]
                    .rearrange("p (r j) -> p r j", j=J)[:, None, :, :]
                    .to_broadcast((P, T_LOC, R, J))
                )
                nc.vector.tensor_tensor(
                    S[:].rearrange("p (t r j) -> p t r j", r=R, j=J),
                    e_in,
                    d_in,
                    mybir.AluOpType.add,
                )
                nc.sync.dma_start(out=out[b, m], in_=S[:])


def build_bass(num_devices=N_CORES):
    key = ("nc", num_devices)
    if key in _CACHE:
        return _CACHE[key]
    import concourse.bacc as bacc
    import concourse.tile as tile
    from concourse import mybir

    nc = bacc.Bacc(
        "TRN2",
        target_bir_lowering=False,
        debug=False,
        num_devices=num_devices,
    )
    f16 = mybir.dt.float16
    aps = {
        "encT": nc.dram_tensor("encT", [D, TOK], f16, kind="ExternalInput").ap(),
        "decT": nc.dram_tensor("decT", [D, BU], f16, kind="ExternalInput").ap(),
        "wencT": nc.dram_tensor("wencT", [D, V], f16, kind="ExternalInput").ap(),
        "wdecT": nc.dram_tensor("wdecT", [D, V], f16, kind="ExternalInput").ap(),
        "out": nc.dram_tensor(
            "out", [B, VT, P, CHUNK], f16, kind="ExternalOutput"
        ).ap(),
    }
    with tile.TileContext(nc) as tc:
        _emit(tc, aps, mybir)
    nc.compile()
    _CACHE[key] = nc
    return nc


def make_in_maps(encoder_outputs, decoder_outputs, fc_weight):
    enc = np.asarray(encoder_outputs, dtype=np.float32)
    dec = np.asarray(decoder_outputs, dtype=np.float32)
    w = np.asarray(fc_weight, dtype=np.float32)
    decT = np.ascontiguousarray(dec.reshape(BU, D).T).astype(np.float16)
    wencT = np.ascontiguousarray(w[:, :D].T).astype(np.float16)
    wdecT = np.ascontiguousarray(w[:, D:].T).astype(np.float16)
    in_maps = []
    for c in range(N_CORES):
        enc_c = enc[:, c * T_LOC : (c + 1) * T_LOC, :].reshape(TOK, D)
        in_maps.append(
            {
                "encT": np.ascontiguousarray(enc_c.T).astype(np.float16),
                "decT": decT,
                "wencT": wencT,
                "wdecT": wdecT,
            }
        )
    return in_maps


def assemble(results):
    """results: list of per-core {"out": (B,VT,P,CHUNK) fp16} -> (B,T,U,V) fp32."""
    full = np.empty((B, T, U, V), dtype=np.float32)
    for c in range(N_CORES):
        arr = results[c]["out"].reshape(B, V, T_LOC, U)
        full[:, c * T_LOC : (c + 1) * T_LOC] = arr.transpose(0, 2, 3, 1)
    return full


def kernel(encoder_outputs, decoder_outputs, fc_weight):
    from concourse.bass_utils import run_bass_kernel_spmd

    nc = build_bass()
    in_maps = make_in_maps(encoder_outputs, decoder_outputs, fc_weight)
    res = run_bass_kernel_spmd(nc, in_maps, list(range(N_CORES)))
    return assemble(res.results)


# revision 5
# speedup vs baseline: 1.9382x; 1.0200x over previous
"""RNN-T joint network kernel for Trainium2 (8 NeuronCores, SPMD).

out[b,t,u,v] = (enc[b,t] @ W_enc.T)[v] + (dec[b,u] @ W_dec.T)[v]

Shapes: enc (4,512,512), dec (4,128,512), W (1024,1024) -> out (4,512,128,1024).

v2 strategy (fp16 output, rel tolerance is 2e-2 so fp16 store is free accuracy-wise):
  - shard T across the 8 cores (64 rows each). Inputs and output in fp16:
    the per-core HBM write drops from 134 MB to 67 MB -> ~187us roofline/core.
  - host pre-transposes inputs to contraction-major fp16; projections are
    fp16 matmuls accumulated in fp32 PSUM, copied to SBUF as fp16 by ACT.
  - broadcast-add as big DVE tensor_tensor fp16 ops [128, 8192] in 2x_1p
    mode (4.33us each): the e-term is pre-replicated only 8x into a small
    e_small[v,(b,t,j8)] tile so BOTH operands keep innermost stride +1
    (access pattern (t, rep16, j8)); the d-term broadcasts along t (outer
    stride 0). DVE total ~150us, under the DMA floor.
  - output written in device layout (B, VT, 128, T_loc*U) fp16, one 2 MB
    DMA per (b, m); host upconverts + transposes when gathering.
"""

import sys

if "/opt/trn_rl_repo" not in sys.path:
    sys.path.insert(0, "/opt/trn_rl_repo")

import numpy as np

# Problem shape (hardcoded per contract)
B, T, U, D, V = 4, 512, 128, 512, 1024
N_CORES = 8
P = 128

T_LOC = T // N_CORES          # 64 t-rows per core
TOK = B * T_LOC               # 256 (b,t) rows per core
KT = D // P                   # 4 contraction tiles
VT = V // P                   # 8 v tiles
BU = B * U                    # 512
CHUNK = T_LOC * U             # 8192 free elems per (b, m) chunk
J = 8                         # e_small replication run
R = U // J                    # 16

_CACHE: dict = {}


def _emit(tc, aps, mybir):
    from contextlib import ExitStack

    nc = tc.nc
    f16 = mybir.dt.float16
    f32 = mybir.dt.float32
    encT, decT, wencT, wdecT, out = (
        aps["encT"], aps["decT"], aps["wencT"], aps["wdecT"], aps["out"],
    )

    with ExitStack() as ctx:
        const = ctx.enter_context(tc.tile_pool(name="const", bufs=1))
        psum = ctx.enter_context(tc.tile_pool(name="psum", bufs=4, space="PSUM"))
        esm = ctx.enter_context(tc.tile_pool(name="esm", bufs=3))
        stage = ctx.enter_context(tc.tile_pool(name="stage", bufs=8))

        # --- input loads, critical-path first ---
        def load(src, lo, hi, tag):
            """One DMA: src[:, lo:hi] (D x w) -> SBUF [P, kt*w], free=(k, col)."""
            w = hi - lo
            t = const.tile([P, KT * w], f16, tag=tag)
            nc.sync.dma_start(
                out=t[:].rearrange("p (k c) -> p k c", c=w),
                in_=src[:, lo:hi].rearrange("(k p) c -> p k c", p=P),
            )
            return t

        wenc_m0 = load(wencT, 0, P, "wenc0")       # [P, 4*128]
        enc_t = load(encT, 0, TOK, "enc")          # [P, 4*256]
        wdec_m0 = load(wdecT, 0, P, "wdec0")       # [P, 4*128]
        dec_t = load(decT, 0, BU, "dec")           # [P, 4*512]
        wenc_r = load(wencT, P, V, "wencr")        # [P, 4*896]
        wdec_r = load(wdecT, P, V, "wdecr")

        def project(lhs, lhs_w, lhs_lo, rhs, rhs_w, n, tag, n0=None):
            """psum[P, n] = sum_k lhs[:, k*lhs_w+lhs_lo : +128].T @ rhs[:, k*rhs_w : +n];
            ACT-copy to SBUF fp16. If n0 is set, the first n0 columns are computed,
            copied, and usable before the rest (shorter critical path for b=0)."""
            ps = psum.tile([P, n], f32, tag="ps" + tag[0])
            sb = const.tile([P, n], f16, tag=tag)
            splits = [(0, n0), (n0, n)] if n0 else [(0, n)]
            for lo, hi in splits:
                for k in range(KT):
                    nc.tensor.matmul(
                        ps[:, lo:hi],
                        lhsT=lhs[:, k * lhs_w + lhs_lo : k * lhs_w + lhs_lo + P],
                        rhs=rhs[:, k * rhs_w + lo : k * rhs_w + hi],
                        start=(k == 0),
                        stop=(k == KT - 1),
                    )
                nc.scalar.activation(
                    sb[:, lo:hi], ps[:, lo:hi], mybir.ActivationFunctionType.Copy
                )
            return sb

        def emit_chunk(es, dproj, b, m, n_pieces):
            """TT + DMA for chunk (b, m), split into n_pieces along t."""
            tw = T_LOC // n_pieces
            for i in range(n_pieces):
                S = stage.tile([P, tw * U], f16, tag="stage")
                t0 = i * tw
                e_in = (
                    es[:, (b * T_LOC + t0) * J : (b * T_LOC + t0 + tw) * J]
                    .rearrange("p (t j) -> p t j", j=J)[:, :, None, :]
                    .to_broadcast((P, tw, R, J))
                )
                d_in = (
                    dproj[:, b * U : (b + 1) * U]
                    .rearrange("p (r j) -> p r j", j=J)[:, None, :, :]
                    .to_broadcast((P, tw, R, J))
                )
                nc.vector.tensor_tensor(
                    S[:].rearrange("p (t r j) -> p t r j", r=R, j=J),
                    e_in,
                    d_in,
                    mybir.AluOpType.add,
                )
                nc.sync.dma_start(
                    out=out[b, m, :, t0 * U : (t0 + tw) * U], in_=S[:]
                )

        for m in range(VT):
            if m == 0:
                we, we_w, we_lo = wenc_m0, P, 0
                wd, wd_w, wd_lo = wdec_m0, P, 0
            else:
                we, we_w, we_lo = wenc_r, V - P, (m - 1) * P
                wd, wd_w, wd_lo = wdec_r, V - P, (m - 1) * P

            # [P,(b,t)] / [P,(b,u)]; for m=0, b=0's columns come out first
            eproj = project(we, we_w, we_lo, enc_t, TOK, TOK, f"eproj{m}",
                            n0=T_LOC if m == 0 else None)
            dproj = project(wd, wd_w, wd_lo, dec_t, BU, BU, f"dproj{m}",
                            n0=U if m == 0 else None)

            # e_small_m [P, (b, t, j)]: e replicated J times along j (DVE 2x copy)
            es = esm.tile([P, B * T_LOC * J], f16, tag="esmall")
            for blo, bhi in ([(0, 1), (1, B)] if m == 0 else [(0, B)]):
                nc.vector.tensor_copy(
                    out=es[:, blo * T_LOC * J : bhi * T_LOC * J].rearrange(
                        "p (b t j) -> p b t j", t=T_LOC, j=J
                    ),
                    in_=eproj[:, blo * T_LOC : bhi * T_LOC]
                    .rearrange("p (b t) -> p b t", t=T_LOC)[:, :, :, None]
                    .to_broadcast((P, bhi - blo, T_LOC, J)),
                )

            # stage[v, (t, r, j)] = e_small[v, (t, j)] bcast over r
            #                      + dproj[v, (r, j)] bcast over t  (all stride+1 innermost)
            # Early m's chunks are split into smaller TT+DMA pieces so the
            # output stream saturates the DMA queues during the ramp.
            n_pieces = 4 if m == 0 else (2 if m == 1 else 1)
            for b in range(B):
                emit_chunk(es, dproj, b, m, n_pieces)


def build_bass(num_devices=N_CORES):
    key = ("nc", num_devices)
    if key in _CACHE:
        return _CACHE[key]
    import concourse.bacc as bacc
    import concourse.tile as tile
    from concourse import mybir

    nc = bacc.Bacc(
        "TRN2",
        target_bir_lowering=False,
        debug=False,
        num_devices=num_devices,
    )
    f16 = mybir.dt.float16
    aps = {
        "encT": nc.dram_tensor("encT", [D, TOK], f16, kind="ExternalInput").ap(),
        "decT": nc.dram_tensor("decT", [D, BU], f16, kind="ExternalInput").ap(),
        "wencT": nc.dram_tensor("wencT", [D, V], f16, kind="ExternalInput").ap(),
        "wdecT": nc.dram_tensor("wdecT", [D, V], f16, kind="ExternalInput").ap(),
        "out": nc.dram_tensor(
            "out", [B, VT, P, CHUNK], f16, kind="ExternalOutput"
        ).ap(),
    }
    with tile.TileContext(nc) as tc:
        _emit(tc, aps, mybir)
    nc.compile()
    _CACHE[key] = nc
    return nc


def make_in_maps(encoder_outputs, decoder_outputs, fc_weight):
    enc = np.asarray(encoder_outputs, dtype=np.float32)
    dec = np.asarray(decoder_outputs, dtype=np.float32)
    w = np.asarray(fc_weight, dtype=np.float32)
    decT = np.ascontiguousarray(dec.reshape(BU, D).T).astype(np.float16)
    wencT = np.ascontiguousarray(w[:, :D].T).astype(np.float16)
    wdecT = np.ascontiguousarray(w[:, D:].T).astype(np.float16)
    in_maps = []
    for c in range(N_CORES):
        enc_c = enc[:, c * T_LOC : (c + 1) * T_LOC, :].reshape(TOK, D)
        in_maps.append(
            {
                "encT": np.ascontiguousarray(enc_c.T).astype(np.float16),
                "decT": decT,
                "wencT": wencT,
                "wdecT": wdecT,
            }
        )
    return in_maps


def assemble(results):
    """results: list of per-core {"out": (B,VT,P,CHUNK) fp16} -> (B,T,U,V) fp32."""
    full = np.empty((B, T, U, V), dtype=np.float32)
    for c in range(N_CORES):
        arr = results[c]["out"].reshape(B, V, T_LOC, U)
        full[:, c * T_LOC : (c + 1) * T_LOC] = arr.transpose(0, 2, 3, 1)
    return full


def kernel(encoder_outputs, decoder_outputs, fc_weight):
    from concourse.bass_utils import run_bass_kernel_spmd

    nc = build_bass()
    in_maps = make_in_maps(encoder_outputs, decoder_outputs, fc_weight)
    res = run_bass_kernel_spmd(nc, in_maps, list(range(N_CORES)))
    return assemble(res.results)


# revision 6
# speedup vs baseline: 2.2125x; 1.1415x over previous
"""RNN-T joint network kernel for Trainium2 (8 NeuronCores, SPMD).

out[b,t,u,v] = (enc[b,t] @ W_enc.T)[v] + (dec[b,u] @ W_dec.T)[v]

Shapes: enc (4,512,512), dec (4,128,512), W (1024,1024) -> out (4,512,128,1024).

v2 strategy (fp16 output, rel tolerance is 2e-2 so fp16 store is free accuracy-wise):
  - shard T across the 8 cores (64 rows each). Inputs and output in fp16:
    the per-core HBM write drops from 134 MB to 67 MB -> ~187us roofline/core.
  - host pre-transposes inputs to contraction-major fp16; projections are
    fp16 matmuls accumulated in fp32 PSUM, copied to SBUF as fp16 by ACT.
  - broadcast-add as big DVE tensor_tensor fp16 ops [128, 8192] in 2x_1p
    mode (4.33us each): the e-term is pre-replicated only 8x into a small
    e_small[v,(b,t,j8)] tile so BOTH operands keep innermost stride +1
    (access pattern (t, rep16, j8)); the d-term broadcasts along t (outer
    stride 0). DVE total ~150us, under the DMA floor.
  - output written in device layout (B, VT, 128, T_loc*U) fp16, one 2 MB
    DMA per (b, m); host upconverts + transposes when gathering.
"""

import sys

if "/opt/trn_rl_repo" not in sys.path:
    sys.path.insert(0, "/opt/trn_rl_repo")

import numpy as np

# Problem shape (hardcoded per contract)
B, T, U, D, V = 4, 512, 128, 512, 1024
N_CORES = 8
P = 128

T_LOC = T // N_CORES          # 64 t-rows per core
TOK = B * T_LOC               # 256 (b,t) rows per core
KT = D // P                   # 4 contraction tiles
VT = V // P                   # 8 v tiles
BU = B * U                    # 512
CHUNK = T_LOC * U             # 8192 free elems per (b, m) chunk
J = 8                         # e_small replication run
R = U // J                    # 16

_CACHE: dict = {}


def _emit(tc, aps, mybir):
    from contextlib import ExitStack

    nc = tc.nc
    f16 = mybir.dt.float16
    f32 = mybir.dt.float32
    encT, decT, wencT, wdecT, out = (
        aps["encT"], aps["decT"], aps["wencT"], aps["wdecT"], aps["out"],
    )

    with ExitStack() as ctx:
        const = ctx.enter_context(tc.tile_pool(name="const", bufs=1))
        psum = ctx.enter_context(tc.tile_pool(name="psum", bufs=4, space="PSUM"))
        esm = ctx.enter_context(tc.tile_pool(name="esm", bufs=3))
        stage = ctx.enter_context(tc.tile_pool(name="stage", bufs=8))

        # --- input loads, critical-path first ---
        def load(src, lo, hi, tag):
            """One DMA on the ACT HWDGE queue (keeps the SP queue free for the
            output stream): src[:, lo:hi] (D x w) -> SBUF [P, kt*w]."""
            w = hi - lo
            t = const.tile([P, KT * w], f16, tag=tag)
            nc.scalar.dma_start(
                out=t[:].rearrange("p (k c) -> p k c", c=w),
                in_=src[:, lo:hi].rearrange("(k p) c -> p k c", p=P),
            )
            return t

        wenc_m0 = load(wencT, 0, P, "wenc0")       # [P, 4*128]
        enc_t = load(encT, 0, TOK, "enc")          # [P, 4*256]
        wdec_m0 = load(wdecT, 0, P, "wdec0")       # [P, 4*128]
        dec_t = load(decT, 0, BU, "dec")           # [P, 4*512]
        wenc_r = load(wencT, P, V, "wencr")        # [P, 4*896]
        wdec_r = load(wdecT, P, V, "wdecr")

        def project(lhs, lhs_w, lhs_lo, rhs, rhs_w, n, tag, n0=None):
            """psum[P, n] = sum_k lhs[:, k*lhs_w+lhs_lo : +128].T @ rhs[:, k*rhs_w : +n];
            ACT-copy to SBUF fp16. If n0 is set, the first n0 columns are computed,
            copied, and usable before the rest (shorter critical path for b=0)."""
            ps = psum.tile([P, n], f32, tag="ps" + tag[0])
            sb = const.tile([P, n], f16, tag=tag)
            splits = [(0, n0), (n0, n)] if n0 else [(0, n)]
            for lo, hi in splits:
                for k in range(KT):
                    nc.tensor.matmul(
                        ps[:, lo:hi],
                        lhsT=lhs[:, k * lhs_w + lhs_lo : k * lhs_w + lhs_lo + P],
                        rhs=rhs[:, k * rhs_w + lo : k * rhs_w + hi],
                        start=(k == 0),
                        stop=(k == KT - 1),
                    )
                nc.scalar.activation(
                    sb[:, lo:hi], ps[:, lo:hi], mybir.ActivationFunctionType.Copy
                )
            return sb

        def emit_chunk(es, dproj, b, m, n_pieces):
            """TT + DMA for chunk (b, m), split into n_pieces along t."""
            tw = T_LOC // n_pieces
            for i in range(n_pieces):
                S = stage.tile([P, tw * U], f16, tag="stage")
                t0 = i * tw
                e_in = (
                    es[:, (b * T_LOC + t0) * J : (b * T_LOC + t0 + tw) * J]
                    .rearrange("p (t j) -> p t j", j=J)[:, :, None, :]
                    .to_broadcast((P, tw, R, J))
                )
                d_in = (
                    dproj[:, b * U : (b + 1) * U]
                    .rearrange("p (r j) -> p r j", j=J)[:, None, :, :]
                    .to_broadcast((P, tw, R, J))
                )
                nc.vector.tensor_tensor(
                    S[:].rearrange("p (t r j) -> p t r j", r=R, j=J),
                    e_in,
                    d_in,
                    mybir.AluOpType.add,
                )
                nc.sync.dma_start(
                    out=out[b, m, :, t0 * U : (t0 + tw) * U], in_=S[:]
                )

        for m in range(VT):
            if m == 0:
                we, we_w, we_lo = wenc_m0, P, 0
                wd, wd_w, wd_lo = wdec_m0, P, 0
            else:
                we, we_w, we_lo = wenc_r, V - P, (m - 1) * P
                wd, wd_w, wd_lo = wdec_r, V - P, (m - 1) * P

            # [P,(b,t)] / [P,(b,u)]; for m=0, b=0's columns come out first
            eproj = project(we, we_w, we_lo, enc_t, TOK, TOK, f"eproj{m}",
                            n0=T_LOC if m == 0 else None)
            dproj = project(wd, wd_w, wd_lo, dec_t, BU, BU, f"dproj{m}",
                            n0=U if m == 0 else None)

            # e_small_m [P, (b, t, j)]: e replicated J times along j (DVE 2x copy)
            es = esm.tile([P, B * T_LOC * J], f16, tag="esmall")
            for blo, bhi in ([(0, 1), (1, B)] if m == 0 else [(0, B)]):
                nc.vector.tensor_copy(
                    out=es[:, blo * T_LOC * J : bhi * T_LOC * J].rearrange(
                        "p (b t j) -> p b t j", t=T_LOC, j=J
                    ),
                    in_=eproj[:, blo * T_LOC : bhi * T_LOC]
                    .rearrange("p (b t) -> p b t", t=T_LOC)[:, :, :, None]
                    .to_broadcast((P, bhi - blo, T_LOC, J)),
                )

            # stage[v, (t, r, j)] = e_small[v, (t, j)] bcast over r
            #                      + dproj[v, (r, j)] bcast over t  (all stride+1 innermost)
            # Early m's chunks are split into smaller TT+DMA pieces so the
            # output stream saturates the DMA queues during the ramp.
            n_pieces = 4 if m == 0 else (2 if m == 1 else 1)
            for b in range(B):
                emit_chunk(es, dproj, b, m, n_pieces)


def build_bass(num_devices=N_CORES):
    key = ("nc", num_devices)
    if key in _CACHE:
        return _CACHE[key]
    import concourse.bacc as bacc
    import concourse.tile as tile
    from concourse import mybir

    nc = bacc.Bacc(
        "TRN2",
        target_bir_lowering=False,
        debug=False,
        num_devices=num_devices,
    )
    f16 = mybir.dt.float16
    aps = {
        "encT": nc.dram_tensor("encT", [D, TOK], f16, kind="ExternalInput").ap(),
        "decT": nc.dram_tensor("decT", [D, BU], f16, kind="ExternalInput").ap(),
        "wencT": nc.dram_tensor("wencT", [D, V], f16, kind="ExternalInput").ap(),
        "wdecT": nc.dram_tensor("wdecT", [D, V], f16, kind="ExternalInput").ap(),
        "out": nc.dram_tensor(
            "out", [B, VT, P, CHUNK], f16, kind="ExternalOutput"
        ).ap(),
    }
    with tile.TileContext(nc) as tc:
        _emit(tc, aps, mybir)
    nc.compile()
    _CACHE[key] = nc
    return nc


def make_in_maps(encoder_outputs, decoder_outputs, fc_weight):
    enc = np.asarray(encoder_outputs, dtype=np.float32)
    dec = np.asarray(decoder_outputs, dtype=np.float32)
    w = np.asarray(fc_weight, dtype=np.float32)
    decT = np.ascontiguousarray(dec.reshape(BU, D).T).astype(np.float16)
    wencT = np.ascontiguousarray(w[:, :D].T).astype(np.float16)
    wdecT = np.ascontiguousarray(w[:, D:].T).astype(np.float16)
    in_maps = []
    for c in range(N_CORES):
        enc_c = enc[:, c * T_LOC : (c + 1) * T_LOC, :].reshape(TOK, D)
        in_maps.append(
            {
                "encT": np.ascontiguousarray(enc_c.T).astype(np.float16),
                "decT": decT,
                "wencT": wencT,
                "wdecT": wdecT,
            }
        )
    return in_maps


def assemble(results):
    """results: list of per-core {"out": (B,VT,P,CHUNK) fp16} -> (B,T,U,V) fp32."""
    full = np.empty((B, T, U, V), dtype=np.float32)
    for c in range(N_CORES):
        arr = results[c]["out"].reshape(B, V, T_LOC, U)
        full[:, c * T_LOC : (c + 1) * T_LOC] = arr.transpose(0, 2, 3, 1)
    return full


def kernel(encoder_outputs, decoder_outputs, fc_weight):
    from concourse.bass_utils import run_bass_kernel_spmd

    nc = build_bass()
    in_maps = make_in_maps(encoder_outputs, decoder_outputs, fc_weight)
    res = run_bass_kernel_spmd(nc, in_maps, list(range(N_CORES)))
    return assemble(res.results)


# revision 10
# speedup vs baseline: 2.2255x; 1.0059x over previous
"""RNN-T joint network kernel for Trainium2 (8 NeuronCores, SPMD).

out[b,t,u,v] = (enc[b,t] @ W_enc.T)[v] + (dec[b,u] @ W_dec.T)[v]

Shapes: enc (4,512,512), dec (4,128,512), W (1024,1024) -> out (4,512,128,1024).

v2 strategy (fp16 output, rel tolerance is 2e-2 so fp16 store is free accuracy-wise):
  - shard T across the 8 cores (64 rows each). Inputs and output in fp16:
    the per-core HBM write drops from 134 MB to 67 MB -> ~187us roofline/core.
  - host pre-transposes inputs to contraction-major fp16; projections are
    fp16 matmuls accumulated in fp32 PSUM, copied to SBUF as fp16 by ACT.
  - broadcast-add as big DVE tensor_tensor fp16 ops [128, 8192] in 2x_1p
    mode (4.33us each): the e-term is pre-replicated only 8x into a small
    e_small[v,(b,t,j8)] tile so BOTH operands keep innermost stride +1
    (access pattern (t, rep16, j8)); the d-term broadcasts along t (outer
    stride 0). DVE total ~150us, under the DMA floor.
  - output written in device layout (B, VT, 128, T_loc*U) fp16, one 2 MB
    DMA per (b, m); host upconverts + transposes when gathering.
"""

import sys

if "/opt/trn_rl_repo" not in sys.path:
    sys.path.insert(0, "/opt/trn_rl_repo")

import numpy as np

# Problem shape (hardcoded per contract)
B, T, U, D, V = 4, 512, 128, 512, 1024
N_CORES = 8
P = 128

T_LOC = T // N_CORES          # 64 t-rows per core
TOK = B * T_LOC               # 256 (b,t) rows per core
KT = D // P                   # 4 contraction tiles
VT = V // P                   # 8 v tiles
BU = B * U                    # 512
CHUNK = T_LOC * U             # 8192 free elems per (b, m) chunk
J = 8                         # e_small replication run
R = U // J                    # 16

_CACHE: dict = {}


def _emit(tc, aps, mybir):
    from contextlib import ExitStack

    nc = tc.nc
    f16 = mybir.dt.float16
    f32 = mybir.dt.float32
    encT, decT, out = aps["encT"], aps["decT"], aps["out"]

    with ExitStack() as ctx:
        const = ctx.enter_context(tc.tile_pool(name="const", bufs=1))
        psum = ctx.enter_context(tc.tile_pool(name="psum", bufs=4, space="PSUM"))
        esm = ctx.enter_context(tc.tile_pool(name="esm", bufs=3))
        stage = ctx.enter_context(tc.tile_pool(name="stage", bufs=8))

        # --- input loads, critical-path first ---
        def load(src, tag):
            """One flat line-rate DMA on the ACT HWDGE queue (keeps the SP
            queue free for the output stream). Host pre-tiles to [P, kt*w]."""
            t = const.tile([P, src.shape[1]], f16, tag=tag)
            nc.scalar.dma_start(out=t[:], in_=src)
            return t

        wenc_m0 = load(aps["wenc0"], "wenc0")      # [P, 4*128]
        enc_t = load(encT, "enc")                  # [P, 4*256]
        wdec_m0 = load(aps["wdec0"], "wdec0")      # [P, 4*128]
        dec_t = load(decT, "dec")                  # [P, 4*512]
        wenc_r = load(aps["wencr"], "wencr")       # [P, 4*896]
        wdec_r = load(aps["wdecr"], "wdecr")

        def project(lhs, lhs_w, lhs_lo, rhs, rhs_w, n, tag, n0=None):
            """psum[P, n] = sum_k lhs[:, k*lhs_w+lhs_lo : +128].T @ rhs[:, k*rhs_w : +n];
            ACT-copy to SBUF fp16. If n0 is set, the first n0 columns are computed,
            copied, and usable before the rest (shorter critical path for b=0)."""
            ps = psum.tile([P, n], f32, tag="ps" + tag[0])
            sb = const.tile([P, n], f16, tag=tag)
            splits = [(0, n0), (n0, n)] if n0 else [(0, n)]
            for lo, hi in splits:
                for k in range(KT):
                    nc.tensor.matmul(
                        ps[:, lo:hi],
                        lhsT=lhs[:, k * lhs_w + lhs_lo : k * lhs_w + lhs_lo + P],
                        rhs=rhs[:, k * rhs_w + lo : k * rhs_w + hi],
                        start=(k == 0),
                        stop=(k == KT - 1),
                    )
                nc.scalar.activation(
                    sb[:, lo:hi], ps[:, lo:hi], mybir.ActivationFunctionType.Copy
                )
            return sb

        def emit_chunk(es, dproj, b, m, n_pieces):
            """TT + DMA for chunk (b, m), split into n_pieces along t."""
            tw = T_LOC // n_pieces
            for i in range(n_pieces):
                S = stage.tile([P, tw * U], f16, tag="stage")
                t0 = i * tw
                e_in = (
                    es[:, (b * T_LOC + t0) * J : (b * T_LOC + t0 + tw) * J]
                    .rearrange("p (t j) -> p t j", j=J)[:, :, None, :]
                    .to_broadcast((P, tw, R, J))
                )
                d_in = (
                    dproj[:, b * U : (b + 1) * U]
                    .rearrange("p (r j) -> p r j", j=J)[:, None, :, :]
                    .to_broadcast((P, tw, R, J))
                )
                nc.vector.tensor_tensor(
                    S[:].rearrange("p (t r j) -> p t r j", r=R, j=J),
                    e_in,
                    d_in,
                    mybir.AluOpType.add,
                )
                nc.sync.dma_start(
                    out=out[b, m, :, t0 * U : (t0 + tw) * U], in_=S[:]
                )

        for m in range(VT):
            if m == 0:
                we, we_w, we_lo = wenc_m0, P, 0
                wd, wd_w, wd_lo = wdec_m0, P, 0
            else:
                we, we_w, we_lo = wenc_r, V - P, (m - 1) * P
                wd, wd_w, wd_lo = wdec_r, V - P, (m - 1) * P

            # [P,(b,t)] / [P,(b,u)]; for m=0, b=0's columns come out first
            eproj = project(we, we_w, we_lo, enc_t, TOK, TOK, f"eproj{m}",
                            n0=T_LOC if m == 0 else None)
            dproj = project(wd, wd_w, wd_lo, dec_t, BU, BU, f"dproj{m}",
                            n0=U if m == 0 else None)

            # e_small_m [P, (b, t, j)]: e replicated J times along j (DVE 2x copy)
            es = esm.tile([P, B * T_LOC * J], f16, tag="esmall")
            for blo, bhi in ([(0, 1), (1, B)] if m == 0 else [(0, B)]):
                nc.vector.tensor_copy(
                    out=es[:, blo * T_LOC * J : bhi * T_LOC * J].rearrange(
                        "p (b t j) -> p b t j", t=T_LOC, j=J
                    ),
                    in_=eproj[:, blo * T_LOC : bhi * T_LOC]
                    .rearrange("p (b t) -> p b t", t=T_LOC)[:, :, :, None]
                    .to_broadcast((P, bhi - blo, T_LOC, J)),
                )

            # stage[v, (t, r, j)] = e_small[v, (t, j)] bcast over r
            #                      + dproj[v, (r, j)] bcast over t  (all stride+1 innermost)
            # Early m's chunks are split into smaller TT+DMA pieces so the
            # output stream saturates the DMA queues during the ramp.
            n_pieces = 4 if m == 0 else (2 if m == 1 else 1)
            for b in range(B):
                emit_chunk(es, dproj, b, m, n_pieces)


def build_bass(num_devices=N_CORES):
    key = ("nc", num_devices)
    if key in _CACHE:
        return _CACHE[key]
    import concourse.bacc as bacc
    import concourse.tile as tile
    from concourse import mybir

    nc = bacc.Bacc(
        "TRN2",
        target_bir_lowering=False,
        debug=False,
        num_devices=num_devices,
    )
    f16 = mybir.dt.float16
    aps = {
        "encT": nc.dram_tensor("encT", [P, KT * TOK], f16, kind="ExternalInput").ap(),
        "decT": nc.dram_tensor("decT", [P, KT * BU], f16, kind="ExternalInput").ap(),
        "wenc0": nc.dram_tensor("wenc0", [P, KT * P], f16, kind="ExternalInput").ap(),
        "wencr": nc.dram_tensor(
            "wencr", [P, KT * (V - P)], f16, kind="ExternalInput"
        ).ap(),
        "wdec0": nc.dram_tensor("wdec0", [P, KT * P], f16, kind="ExternalInput").ap(),
        "wdecr": nc.dram_tensor(
            "wdecr", [P, KT * (V - P)], f16, kind="ExternalInput"
        ).ap(),
        "out": nc.dram_tensor(
            "out", [B, VT, P, CHUNK], f16, kind="ExternalOutput"
        ).ap(),
    }
    with tile.TileContext(nc) as tc:
        _emit(tc, aps, mybir)
    nc.compile()
    _CACHE[key] = nc
    return nc


def _tile_kmajor(srcT):
    """[D, w] contraction-major -> [P, KT*w] pre-tiled for a flat SBUF load."""
    w = srcT.shape[1]
    return np.ascontiguousarray(
        srcT.reshape(KT, P, w).transpose(1, 0, 2).reshape(P, KT * w)
    ).astype(np.float16)


def make_in_maps(encoder_outputs, decoder_outputs, fc_weight):
    enc = np.asarray(encoder_outputs, dtype=np.float32)
    dec = np.asarray(decoder_outputs, dtype=np.float32)
    w = np.asarray(fc_weight, dtype=np.float32)
    decT = _tile_kmajor(dec.reshape(BU, D).T)
    wencT = w[:, :D].T  # [D, V]
    wdecT = w[:, D:].T
    wenc0, wencr = _tile_kmajor(wencT[:, :P]), _tile_kmajor(wencT[:, P:])
    wdec0, wdecr = _tile_kmajor(wdecT[:, :P]), _tile_kmajor(wdecT[:, P:])
    in_maps = []
    for c in range(N_CORES):
        enc_c = enc[:, c * T_LOC : (c + 1) * T_LOC, :].reshape(TOK, D)
        in_maps.append(
            {
                "encT": _tile_kmajor(enc_c.T),
                "decT": decT,
                "wenc0": wenc0,
                "wencr": wencr,
                "wdec0": wdec0,
                "wdecr": wdecr,
            }
        )
    return in_maps


def assemble(results):
    """results: list of per-core {"out": (B,VT,P,CHUNK) fp16} -> (B,T,U,V) fp32."""
    full = np.empty((B, T, U, V), dtype=np.float32)
    for c in range(N_CORES):
        arr = results[c]["out"].reshape(B, V, T_LOC, U)
        full[:, c * T_LOC : (c + 1) * T_LOC] = arr.transpose(0, 2, 3, 1)
    return full


def kernel(encoder_outputs, decoder_outputs, fc_weight):
    from concourse.bass_utils import run_bass_kernel_spmd

    nc = build_bass()
    in_maps = make_in_maps(encoder_outputs, decoder_outputs, fc_weight)
    res = run_bass_kernel_spmd(nc, in_maps, list(range(N_CORES)))
    return assemble(res.results)
